# revision 17
# baseline (speedup 1.0000x reference)
"""Trainium2 Bass kernel for nn_MultiHeadAttention_56375740727430.

Causal multi-head attention, B=2 S=2048 D=1024 H=16 KS=64; the final
`heads @ kernel` projection plus softmax normalization run on the host.

Sharding: pure data/head parallel over 8 cores — core c handles batch c//4
and 4 heads (c%4)*4 ... +4, as two head-pairs.  Each core computes Q^T/K^T
(pair-stacked, transposed layout), V (natural layout with an appended
ones-column so the softmax denominator Z rides along the P@V matmul), then
causal scores -> exp -> (P@V | Z), all unnormalized.  The raw [KS+1, S]
o-tensors stream back to DRAM; the host divides by Z, applies the output
projection, and sums head/batch contributions.

Schedule notes (from HW traces/microbenches):
 - emission is software-pipelined: each block's scores are emitted before
   the previous block's P@V, so the scalar engine (exp — the attention
   bottleneck) always has a block of work queued and never starves at
   block boundaries.
 - scores (K=64) run as row-group-alternating pairs: 64-row tile mode
   streams 2 cols/cycle and the alternation hides LDWEIGHTS.
 - all scores of a query block are emitted before its P@V matmuls:
   64-row <-> 128-row tile-mode switches drain the PE (~190ns), so they
   happen once per block instead of twice per key-tile.
 - P@V stationary operands are 128-column windows of V (63 junk columns
   land in PSUM partitions 65..127 and are never read) so Fast Weight
   Load stays enabled; a 65-column weight measured ~330ns/mm vs ~215.
 - PSUM->SBUF copies run on the scalar engine early (while exp has not
   started) and on the vector engine once attention is underway; big
   DMAs alternate between the two HWDGE queues (sync / scalar).
"""

import sys

sys.path.insert(0, "/opt/trn_rl_repo")

from contextlib import ExitStack

import ml_dtypes
import numpy as np

import concourse.bass as bass
import concourse.bacc as bacc
import concourse.mybir as mybir
import concourse.tile as tile

B, S, D = 2, 2048, 1024
H, KS = 16, 64

P = 128            # partitions
NCORES = 8
CORES_PER_B = NCORES // B          # 4
NH = H // CORES_PER_B              # heads per core = 4
NW = NH * KS                       # per-core projection width = 256
DT = D // P                        # d-tiles = 8
ST = S // P                        # s/l-tiles = 16
IB = 512                           # query block
NIB = S // IB                      # 4
LPB = IB // P                      # l-tiles per query block = 4
VS = KS + 8                        # per-head V stride: 64 V + ones + pad (16B-aligned)
VW = NH * VS + 64                  # v row width incl. FWL window pad

F32 = mybir.dt.float32
BF16 = mybir.dt.bfloat16
NP_BF16 = ml_dtypes.bfloat16
EXP = mybir.ActivationFunctionType.Exp


def build_nc():
    mm_dt = BF16
    nc = bacc.Bacc()

    xT = nc.declare_dram_parameter("xT", [D, S], mm_dt, isOutput=False)
    wq = nc.declare_dram_parameter("wq", [D, NW], mm_dt, isOutput=False)
    wk = nc.declare_dram_parameter("wk", [D, NW], mm_dt, isOutput=False)
    wv = nc.declare_dram_parameter("wv", [D, NW], mm_dt, isOutput=False)
    masks = nc.declare_dram_parameter("masks", [P, P], mm_dt, isOutput=False)
    o = nc.declare_dram_parameter("o", [NH, KS + 1, S], BF16, isOutput=True)

    with tile.TileContext(nc) as tc, ExitStack() as ctx:
        const_pool = ctx.enter_context(tc.tile_pool(name="const", bufs=1))
        xw_pool = ctx.enter_context(tc.tile_pool(name="xw", bufs=1))
        qkv_pool = ctx.enter_context(tc.tile_pool(name="qkv", bufs=1))
        pe_pool = ctx.enter_context(tc.tile_pool(name="pexp", bufs=36))
        osb_pool = ctx.enter_context(tc.tile_pool(name="osb", bufs=3))
        pp = ctx.enter_context(
            tc.tile_pool(name="pproj", bufs=2, space=bass.MemorySpace.PSUM)
        )
        pst = ctx.enter_context(
            tc.tile_pool(name="pst", bufs=2, space=bass.MemorySpace.PSUM)
        )
        po = ctx.enter_context(
            tc.tile_pool(name="po", bufs=2, space=bass.MemorySpace.PSUM)
        )

        # ---- constant tiles + memsets first (no deps, run at t~0;
        # warm_in first so PE warmup is unblocked earliest) ----
        mask_sb = const_pool.tile([P, P], mm_dt)
        warm_in = const_pool.tile([P, IB], mm_dt)
        nc.vector.memset(warm_in[:], 0.0)
        dummy = const_pool.tile([P, 2, 2], mm_dt)
        nc.vector.memset(dummy[:, 0, :], 0.0)
        v_sb = qkv_pool.tile([P, ST, VW], mm_dt, tag="v")
        nc.vector.memset(
            v_sb[:, :, 0 : NH * VS]
            .rearrange("p s (h c) -> p s h c", c=VS)[:, :, :, KS:VS],
            1.0,
        )
        nc.vector.memset(v_sb[:, :, NH * VS : VW], 0.0)  # FWL window pad

        # ---- input DMAs, in first-use order, across both HWDGE queues.
        # These must precede the exp-table preload: the scalar-side DMA
        # triggers share the Activation engine queue with it, and the
        # ~2.7us table load would delay half the input chunks ----
        dmae = [nc.sync, nc.sync]
        w_sb = {}
        for name, wh in (("q", wq), ("k", wk)):
            w_sb[name] = xw_pool.tile([P, DT, NW], mm_dt, tag=f"w{name}", name=f"w{name}")
        dmae[0].dma_start(w_sb["q"][:], wq[:].rearrange("(t p) n -> p t n", p=P))
        xT_sb = xw_pool.tile([P, DT, S], mm_dt, tag="xT")
        for t in range(DT):
            dmae[t % 2].dma_start(
                xT_sb[:, t, 0:IB], xT[t * P : (t + 1) * P, 0:IB]
            )
        dmae[1].dma_start(w_sb["k"][:], wk[:].rearrange("(t p) n -> p t n", p=P))
        w_sb["v"] = xw_pool.tile([P, DT, NW], mm_dt, tag="wv", name="wv")
        dmae[0].dma_start(w_sb["v"][:], wv[:].rearrange("(t p) n -> p t n", p=P))
        dmae[0].dma_start(mask_sb[:], masks[:])
        for ic in range(1, NIB):
            for t in range(DT):
                dmae[(ic + t) % 2].dma_start(
                    xT_sb[:, t, ic * IB : (ic + 1) * IB],
                    xT[t * P : (t + 1) * P, ic * IB : (ic + 1) * IB],
                )

        # ---- ACT exp-table preload (~2.7us) during the DMA lead-in ----
        nc.scalar.activation(dummy[:, 1, :], dummy[:, 0, :], EXP)

        qt_sb = [
            qkv_pool.tile([P, S], mm_dt, tag=f"qt{i}", name=f"qt{i}") for i in range(2)
        ]
        kt_sb = [
            qkv_pool.tile([P, S], mm_dt, tag=f"kt{i}", name=f"kt{i}") for i in range(2)
        ]

        # ---- PE warmup: dependency-free matmuls so the HAM clock gate
        # reaches 8/8 during the input-DMA lead-in ----
        for _ in range(8):
            w_ps = pp.tile([P, IB], F32, tag="pp", name="warm")
            nc.tensor.matmul(
                w_ps[:], warm_in[:, 0:P], warm_in[:], start=True, stop=True
            )

        def proj_qk_chunk(pr, ic, eng):
            # qt/kt columns [ic*IB, (ic+1)*IB) for head pair pr
            for wname, dst in (("q", qt_sb), ("k", kt_sb)):
                ps = pp.tile([P, IB], F32, tag="pp", name=f"p{wname}{pr}{ic}")
                for t in range(DT):
                    nc.tensor.matmul(
                        ps[:],
                        w_sb[wname][:, t, pr * P : (pr + 1) * P],
                        xT_sb[:, t, ic * IB : (ic + 1) * IB],
                        start=(t == 0),
                        stop=(t == DT - 1),
                    )
                eng(dst[pr][:, ic * IB : (ic + 1) * IB], ps[:])

        def proj_v_chunk(g, eng):
            # V s-tiles [4g, 4g+4), natural layout, all heads
            for st in range(4 * g, 4 * g + 4):
                ps = pp.tile([P, NW], F32, tag="pp", name=f"pv{st}")
                for t in range(DT):
                    nc.tensor.matmul(
                        ps[:],
                        xT_sb[:, t, st * P : (st + 1) * P],
                        w_sb["v"][:, t, :],
                        start=(t == 0),
                        stop=(t == DT - 1),
                    )
                eng(
                    v_sb[:, st, 0 : NH * VS]
                    .rearrange("p (h c) -> p h c", c=VS)[:, :, 0:KS],
                    ps[:].rearrange("p (h k) -> p h k", k=KS),
                )

        def att_scores(pr, ib):
            # scores -> exp -> mask for head pair pr, query block ib;
            # all in 64-row tile mode, row-group-alternating pairs
            nl = (ib + 1) * LPB
            pes = []
            for lt in range(nl):
                off = max(0, (lt - ib * LPB)) * P
                st = pst.tile([P, 2, IB], F32, tag="st", name=f"st{pr}{ib}{lt}")
                for hh in range(2):
                    nc.tensor.matmul(
                        st[:, hh, off:IB],
                        kt_sb[pr][hh * KS : (hh + 1) * KS, lt * P : (lt + 1) * P],
                        qt_sb[pr][
                            hh * KS : (hh + 1) * KS,
                            ib * IB + off : (ib + 1) * IB,
                        ],
                        start=True,
                        stop=True,
                        tile_position=(hh * KS, 0),
                    )
                pe_t = pe_pool.tile([P, 2, IB], BF16, tag="pe", name=f"pe{pr}{ib}{lt}")
                nc.scalar.activation(
                    pe_t[:, :, off:IB], st[:, :, off:IB], EXP, scale=0.125
                )
                if lt >= ib * LPB:  # diagonal 128-block -> triangular mask
                    for hh in range(2):
                        nc.vector.tensor_mul(
                            pe_t[:, hh, off : off + P],
                            pe_t[:, hh, off : off + P],
                            mask_sb[:],
                        )
                pes.append((pe_t, off))
            return pes

        def att_pv(pr, ib, pes, pool=None, eng=None):
            # P@V (128-row mode); stationary V windows are 128 cols wide
            # to keep FWL on — output rows 65..127 are junk, never read.
            # The final blocks alternate between the po and pp pools so
            # the tail P@V chains don't stall on o-copy drains.
            nl = (ib + 1) * LPB
            pool = pool or po
            eng = eng or nc.vector.tensor_copy
            for hh in range(2):
                o_ps = pool.tile(
                    [P, IB], F32,
                    tag="o" if pool is po else "pp",
                    name=f"o{pr}{ib}{hh}",
                )
                base = (2 * pr + hh) * VS
                for lt in range(nl):
                    pe_t, off = pes[lt]
                    nc.tensor.matmul(
                        o_ps[:, off:IB],
                        v_sb[:, lt, base : base + P],
                        pe_t[:, hh, off:IB],
                        start=(lt == 0),
                        stop=(lt == nl - 1),
                    )
                o_sb = osb_pool.tile(
                    [KS + 1, IB], BF16, tag="osb", name=f"osb{pr}{ib}{hh}"
                )
                eng(o_sb[:], o_ps[0 : KS + 1, :])
                dmae[hh].dma_start(
                    o[2 * pr + hh, :, ib * IB : (ib + 1) * IB], o_sb[:]
                )

        # ---- main schedule: 2-deep software pipeline — each block's
        # scores are emitted before the previous block's P@V so the
        # scalar engine always has exp work queued; the smallest block
        # (1,0) runs last to shorten the PE tail ----
        sc, ve = nc.scalar.copy, nc.vector.tensor_copy
        pes = {}

        def blk_s(pr, ib):
            pes[(pr, ib)] = att_scores(pr, ib)

        def blk_pv(pr, ib, pool=None, eng=None):
            att_pv(pr, ib, pes.pop((pr, ib)), pool=pool, eng=eng)

        proj_qk_chunk(0, 0, sc); blk_s(0, 0)
        proj_qk_chunk(0, 1, sc); blk_s(0, 1)
        proj_v_chunk(0, sc)
        proj_qk_chunk(0, 2, sc); blk_s(0, 2)
        blk_pv(0, 0)
        proj_v_chunk(1, ve)
        proj_qk_chunk(0, 3, sc); blk_s(0, 3)
        blk_pv(0, 1)
        proj_v_chunk(2, ve)
        proj_qk_chunk(1, 0, ve)
        proj_qk_chunk(1, 1, ve); blk_s(1, 1)
        blk_pv(0, 2)
        proj_v_chunk(3, ve)
        proj_qk_chunk(1, 2, ve); blk_s(1, 2)
        blk_pv(0, 3)
        proj_qk_chunk(1, 3, ve); blk_s(1, 3)
        blk_pv(1, 1)
        blk_s(1, 0)
        blk_pv(1, 2)
        blk_pv(1, 3, pool=pp, eng=nc.scalar.copy)
        blk_pv(1, 0, eng=nc.scalar.copy)

    nc.compile()
    return nc


def make_masks():
    # triangular [P, P]: within a diagonal 128-block keep j >= p
    j = np.arange(P)[None, :]
    p = np.arange(P)[:, None]
    return (j >= p).astype(NP_BF16)


def make_in_maps(inputs):
    x = np.asarray(inputs["x"], np.float32)
    Wq = np.asarray(inputs["Wq"], np.float32)
    Wk = np.asarray(inputs["Wk"], np.float32)
    Wv = np.asarray(inputs["Wv"], np.float32)

    masks = make_masks()
    in_maps = []
    for c in range(NCORES):
        b, hs = c // CORES_PER_B, (c % CORES_PER_B) * NH
        in_maps.append(
            {
                "xT": x[b].T.astype(NP_BF16),
                "wq": Wq[:, :, hs : hs + NH].transpose(0, 2, 1).reshape(D, NW)
                .astype(NP_BF16),
                "wk": Wk[:, :, hs : hs + NH].transpose(0, 2, 1).reshape(D, NW)
                .astype(NP_BF16),
                "wv": Wv[:, :, hs : hs + NH].transpose(0, 2, 1).reshape(D, NW)
                .astype(NP_BF16),
                "masks": masks,
            }
        )
    return in_maps


def gather_output(results, kern):
    # normalize by Z, apply the output projection, sum heads + batches
    kern3 = np.asarray(kern, np.float32).reshape(KS, H, KS)  # [k, h, j]
    out = np.zeros((B, S, KS), np.float32)
    for c in range(NCORES):
        b, hs = c // CORES_PER_B, (c % CORES_PER_B) * NH
        oarr = np.asarray(results[c]["o"], np.float32)  # [NH, KS+1, S]
        for hh in range(NH):
            h = hs + hh
            z = oarr[hh, KS, :]
            heads = (oarr[hh, :KS, :] / z[None, :]).T  # [S, KS]
            out[b] += heads @ kern3[:, h, :]
    return out


_NC_CACHE = {}


def get_nc():
    if "nc" not in _NC_CACHE:
        _NC_CACHE["nc"] = build_nc()
    return _NC_CACHE["nc"]


def run_hw(inputs, trace=False, **kw):
    from concourse.bass_utils import run_bass_kernel_spmd

    nc = get_nc()
    in_maps = make_in_maps(inputs)
    res = run_bass_kernel_spmd(
        nc, in_maps, list(range(NCORES)), trace=trace, **kw
    )
    return gather_output(res.results, inputs["kernel"]), res


def kernel(**inputs) -> np.ndarray:
    out, _ = run_hw(inputs, trace=False)
    return out


# revision 18
# speedup vs baseline: 1.0169x; 1.0169x over previous
"""Trainium2 Bass kernel for nn_MultiHeadAttention_56375740727430.

Causal multi-head attention, B=2 S=2048 D=1024 H=16 KS=64; the final
`heads @ kernel` projection plus softmax normalization run on the host.

Sharding: pure data/head parallel over 8 cores — core c handles batch c//4
and 4 heads (c%4)*4 ... +4, as two head-pairs.  Each core computes Q^T/K^T
(pair-stacked, transposed layout), V (natural layout with an appended
ones-column so the softmax denominator Z rides along the P@V matmul), then
causal scores -> exp -> (P@V | Z), all unnormalized.  The raw [KS+1, S]
o-tensors stream back to DRAM; the host divides by Z, applies the output
projection, and sums head/batch contributions.

Schedule notes (from HW traces/microbenches):
 - emission is software-pipelined: each block's scores are emitted before
   the previous block's P@V, so the scalar engine (exp — the attention
   bottleneck) always has a block of work queued and never starves at
   block boundaries.
 - scores (K=64) run as row-group-alternating pairs: 64-row tile mode
   streams 2 cols/cycle and the alternation hides LDWEIGHTS.
 - all scores of a query block are emitted before its P@V matmuls:
   64-row <-> 128-row tile-mode switches drain the PE (~190ns), so they
   happen once per block instead of twice per key-tile.
 - P@V stationary operands are 128-column, 16B-aligned windows of V
   (junk columns land in PSUM partitions 65..127, never read) so Fast
   Weight Load stays enabled.
 - PSUM->SBUF copies run on the scalar engine while exp has not started
   (early projection chunks, final o-blocks) and on the vector engine
   otherwise.  All bulk DMA stays on the sync HWDGE queue — splitting
   input loads across both HWDGE queues measured ~3x slower transfers.
 - o streams back as bf16 (host normalizes in fp32); the final two P@V
   blocks borrow the proj/po PSUM pools so the tail never stalls on
   o-copy drains.
"""

import sys

sys.path.insert(0, "/opt/trn_rl_repo")

from contextlib import ExitStack

import ml_dtypes
import numpy as np

import concourse.bass as bass
import concourse.bacc as bacc
import concourse.mybir as mybir
import concourse.tile as tile

B, S, D = 2, 2048, 1024
H, KS = 16, 64

P = 128            # partitions
NCORES = 8
CORES_PER_B = NCORES // B          # 4
NH = H // CORES_PER_B              # heads per core = 4
NW = NH * KS                       # per-core projection width = 256
DT = D // P                        # d-tiles = 8
ST = S // P                        # s/l-tiles = 16
IB = 512                           # query block
NIB = S // IB                      # 4
LPB = IB // P                      # l-tiles per query block = 4
VS = KS + 8                        # per-head V stride: 64 V + ones + pad (16B-aligned)
VW = NH * VS + 64                  # v row width incl. FWL window pad

F32 = mybir.dt.float32
BF16 = mybir.dt.bfloat16
NP_BF16 = ml_dtypes.bfloat16
EXP = mybir.ActivationFunctionType.Exp


def build_nc():
    mm_dt = BF16
    nc = bacc.Bacc()

    xT = nc.declare_dram_parameter("xT", [D, S], mm_dt, isOutput=False)
    wq = nc.declare_dram_parameter("wq", [D, NW], mm_dt, isOutput=False)
    wk = nc.declare_dram_parameter("wk", [D, NW], mm_dt, isOutput=False)
    wv = nc.declare_dram_parameter("wv", [D, NW], mm_dt, isOutput=False)
    masks = nc.declare_dram_parameter("masks", [P, P], mm_dt, isOutput=False)
    o = nc.declare_dram_parameter("o", [NH, KS + 1, S], BF16, isOutput=True)

    with tile.TileContext(nc) as tc, ExitStack() as ctx:
        const_pool = ctx.enter_context(tc.tile_pool(name="const", bufs=1))
        xw_pool = ctx.enter_context(tc.tile_pool(name="xw", bufs=1))
        qkv_pool = ctx.enter_context(tc.tile_pool(name="qkv", bufs=1))
        pe_pool = ctx.enter_context(tc.tile_pool(name="pexp", bufs=36))
        osb_pool = ctx.enter_context(tc.tile_pool(name="osb", bufs=3))
        pp = ctx.enter_context(
            tc.tile_pool(name="pproj", bufs=2, space=bass.MemorySpace.PSUM)
        )
        pst = ctx.enter_context(
            tc.tile_pool(name="pst", bufs=2, space=bass.MemorySpace.PSUM)
        )
        po = ctx.enter_context(
            tc.tile_pool(name="po", bufs=2, space=bass.MemorySpace.PSUM)
        )

        # ---- constant tiles + memsets first (no deps, run at t~0;
        # warm_in first so PE warmup is unblocked earliest) ----
        mask_sb = const_pool.tile([P, P], mm_dt)
        warm_in = const_pool.tile([P, IB], mm_dt)
        nc.vector.memset(warm_in[:], 0.0)
        dummy = const_pool.tile([P, 2, 2], mm_dt)
        nc.vector.memset(dummy[:, 0, :], 0.0)
        v_sb = qkv_pool.tile([P, ST, VW], mm_dt, tag="v")
        nc.vector.memset(
            v_sb[:, :, 0 : NH * VS]
            .rearrange("p s (h c) -> p s h c", c=VS)[:, :, :, KS:VS],
            1.0,
        )
        nc.vector.memset(v_sb[:, :, NH * VS : VW], 0.0)  # FWL window pad

        # ---- input DMAs, in first-use order, across both HWDGE queues.
        # These must precede the exp-table preload: the scalar-side DMA
        # triggers share the Activation engine queue with it, and the
        # ~2.7us table load would delay half the input chunks ----
        dmae = [nc.sync, nc.sync]
        w_sb = {}
        for name, wh in (("q", wq), ("k", wk)):
            w_sb[name] = xw_pool.tile([P, DT, NW], mm_dt, tag=f"w{name}", name=f"w{name}")
        dmae[0].dma_start(w_sb["q"][:], wq[:].rearrange("(t p) n -> p t n", p=P))
        xT_sb = xw_pool.tile([P, DT, S], mm_dt, tag="xT")
        for t in range(DT):
            dmae[t % 2].dma_start(
                xT_sb[:, t, 0:IB], xT[t * P : (t + 1) * P, 0:IB]
            )
        dmae[1].dma_start(w_sb["k"][:], wk[:].rearrange("(t p) n -> p t n", p=P))
        w_sb["v"] = xw_pool.tile([P, DT, NW], mm_dt, tag="wv", name="wv")
        dmae[0].dma_start(w_sb["v"][:], wv[:].rearrange("(t p) n -> p t n", p=P))
        dmae[0].dma_start(mask_sb[:], masks[:])
        for ic in range(1, NIB):
            for t in range(DT):
                dmae[(ic + t) % 2].dma_start(
                    xT_sb[:, t, ic * IB : (ic + 1) * IB],
                    xT[t * P : (t + 1) * P, ic * IB : (ic + 1) * IB],
                )

        # ---- ACT exp-table preload (~2.7us) during the DMA lead-in ----
        nc.scalar.activation(dummy[:, 1, :], dummy[:, 0, :], EXP)

        qt_sb = [
            qkv_pool.tile([P, S], mm_dt, tag=f"qt{i}", name=f"qt{i}") for i in range(2)
        ]
        kt_sb = [
            qkv_pool.tile([P, S], mm_dt, tag=f"kt{i}", name=f"kt{i}") for i in range(2)
        ]

        # ---- PE warmup: dependency-free matmuls so the HAM clock gate
        # reaches 8/8 during the input-DMA lead-in ----
        for _ in range(14):
            w_ps = pp.tile([P, IB], F32, tag="pp", name="warm")
            nc.tensor.matmul(
                w_ps[:], warm_in[:, 0:P], warm_in[:], start=True, stop=True
            )

        def proj_qk_chunk(pr, ic, eng):
            # qt/kt columns [ic*IB, (ic+1)*IB) for head pair pr
            for wname, dst in (("q", qt_sb), ("k", kt_sb)):
                ps = pp.tile([P, IB], F32, tag="pp", name=f"p{wname}{pr}{ic}")
                for t in range(DT):
                    nc.tensor.matmul(
                        ps[:],
                        w_sb[wname][:, t, pr * P : (pr + 1) * P],
                        xT_sb[:, t, ic * IB : (ic + 1) * IB],
                        start=(t == 0),
                        stop=(t == DT - 1),
                    )
                eng(dst[pr][:, ic * IB : (ic + 1) * IB], ps[:])

        def proj_v_chunk(g, eng):
            # V s-tiles [4g, 4g+4), natural layout, all heads
            for st in range(4 * g, 4 * g + 4):
                ps = pp.tile([P, NW], F32, tag="pp", name=f"pv{st}")
                for t in range(DT):
                    nc.tensor.matmul(
                        ps[:],
                        xT_sb[:, t, st * P : (st + 1) * P],
                        w_sb["v"][:, t, :],
                        start=(t == 0),
                        stop=(t == DT - 1),
                    )
                eng(
                    v_sb[:, st, 0 : NH * VS]
                    .rearrange("p (h c) -> p h c", c=VS)[:, :, 0:KS],
                    ps[:].rearrange("p (h k) -> p h k", k=KS),
                )

        def att_scores(pr, ib):
            # scores -> exp -> mask for head pair pr, query block ib;
            # all in 64-row tile mode, row-group-alternating pairs
            nl = (ib + 1) * LPB
            pes = []
            for lt in range(nl):
                off = max(0, (lt - ib * LPB)) * P
                st = pst.tile([P, 2, IB], F32, tag="st", name=f"st{pr}{ib}{lt}")
                for hh in range(2):
                    nc.tensor.matmul(
                        st[:, hh, off:IB],
                        kt_sb[pr][hh * KS : (hh + 1) * KS, lt * P : (lt + 1) * P],
                        qt_sb[pr][
                            hh * KS : (hh + 1) * KS,
                            ib * IB + off : (ib + 1) * IB,
                        ],
                        start=True,
                        stop=True,
                        tile_position=(hh * KS, 0),
                    )
                pe_t = pe_pool.tile([P, 2, IB], BF16, tag="pe", name=f"pe{pr}{ib}{lt}")
                nc.scalar.activation(
                    pe_t[:, :, off:IB], st[:, :, off:IB], EXP, scale=0.125
                )
                if lt >= ib * LPB:  # diagonal 128-block -> triangular mask
                    for hh in range(2):
                        nc.vector.tensor_mul(
                            pe_t[:, hh, off : off + P],
                            pe_t[:, hh, off : off + P],
                            mask_sb[:],
                        )
                pes.append((pe_t, off))
            return pes

        def att_pv(pr, ib, pes, pool=None, eng=None):
            # P@V (128-row mode); stationary V windows are 128 cols wide
            # to keep FWL on — output rows 65..127 are junk, never read.
            # The final blocks alternate between the po and pp pools so
            # the tail P@V chains don't stall on o-copy drains.
            nl = (ib + 1) * LPB
            pool = pool or po
            eng = eng or nc.vector.tensor_copy
            for hh in range(2):
                o_ps = pool.tile(
                    [P, IB], F32,
                    tag="o" if pool is po else "pp",
                    name=f"o{pr}{ib}{hh}",
                )
                base = (2 * pr + hh) * VS
                for lt in range(nl):
                    pe_t, off = pes[lt]
                    nc.tensor.matmul(
                        o_ps[:, off:IB],
                        v_sb[:, lt, base : base + P],
                        pe_t[:, hh, off:IB],
                        start=(lt == 0),
                        stop=(lt == nl - 1),
                    )
                o_sb = osb_pool.tile(
                    [KS + 1, IB], BF16, tag="osb", name=f"osb{pr}{ib}{hh}"
                )
                eng(o_sb[:], o_ps[0 : KS + 1, :])
                dmae[hh].dma_start(
                    o[2 * pr + hh, :, ib * IB : (ib + 1) * IB], o_sb[:]
                )

        # ---- main schedule: 2-deep software pipeline — each block's
        # scores are emitted before the previous block's P@V so the
        # scalar engine always has exp work queued; the smallest block
        # (1,0) runs last to shorten the PE tail ----
        sc, ve = nc.scalar.copy, nc.vector.tensor_copy
        pes = {}

        def blk_s(pr, ib):
            pes[(pr, ib)] = att_scores(pr, ib)

        def blk_pv(pr, ib, pool=None, eng=None):
            att_pv(pr, ib, pes.pop((pr, ib)), pool=pool, eng=eng)

        proj_qk_chunk(0, 0, sc); blk_s(0, 0)
        proj_qk_chunk(0, 1, sc); blk_s(0, 1)
        proj_v_chunk(0, sc)
        proj_qk_chunk(0, 2, sc); blk_s(0, 2)
        blk_pv(0, 0)
        proj_v_chunk(1, ve)
        proj_qk_chunk(0, 3, sc); blk_s(0, 3)
        blk_pv(0, 1)
        proj_v_chunk(2, ve)
        proj_qk_chunk(1, 0, ve)
        proj_qk_chunk(1, 1, ve); blk_s(1, 1)
        blk_pv(0, 2)
        proj_v_chunk(3, ve)
        proj_qk_chunk(1, 2, ve); blk_s(1, 2)
        blk_pv(0, 3)
        proj_qk_chunk(1, 3, ve); blk_s(1, 3)
        blk_pv(1, 1)
        blk_s(1, 0)
        blk_pv(1, 2)
        blk_pv(1, 3, pool=pp, eng=nc.scalar.copy)
        blk_pv(1, 0, eng=nc.scalar.copy)

    nc.compile()
    return nc


def make_masks():
    # triangular [P, P]: within a diagonal 128-block keep j >= p
    j = np.arange(P)[None, :]
    p = np.arange(P)[:, None]
    return (j >= p).astype(NP_BF16)


def make_in_maps(inputs):
    x = np.asarray(inputs["x"], np.float32)
    Wq = np.asarray(inputs["Wq"], np.float32)
    Wk = np.asarray(inputs["Wk"], np.float32)
    Wv = np.asarray(inputs["Wv"], np.float32)

    masks = make_masks()
    in_maps = []
    for c in range(NCORES):
        b, hs = c // CORES_PER_B, (c % CORES_PER_B) * NH
        in_maps.append(
            {
                "xT": x[b].T.astype(NP_BF16),
                "wq": Wq[:, :, hs : hs + NH].transpose(0, 2, 1).reshape(D, NW)
                .astype(NP_BF16),
                "wk": Wk[:, :, hs : hs + NH].transpose(0, 2, 1).reshape(D, NW)
                .astype(NP_BF16),
                "wv": Wv[:, :, hs : hs + NH].transpose(0, 2, 1).reshape(D, NW)
                .astype(NP_BF16),
                "masks": masks,
            }
        )
    return in_maps


def gather_output(results, kern):
    # normalize by Z, apply the output projection, sum heads + batches
    kern3 = np.asarray(kern, np.float32).reshape(KS, H, KS)  # [k, h, j]
    out = np.zeros((B, S, KS), np.float32)
    for c in range(NCORES):
        b, hs = c // CORES_PER_B, (c % CORES_PER_B) * NH
        oarr = np.asarray(results[c]["o"], np.float32)  # [NH, KS+1, S]
        for hh in range(NH):
            h = hs + hh
            z = oarr[hh, KS, :]
            heads = (oarr[hh, :KS, :] / z[None, :]).T  # [S, KS]
            out[b] += heads @ kern3[:, h, :]
    return out


_NC_CACHE = {}


def get_nc():
    if "nc" not in _NC_CACHE:
        _NC_CACHE["nc"] = build_nc()
    return _NC_CACHE["nc"]


def run_hw(inputs, trace=False, **kw):
    from concourse.bass_utils import run_bass_kernel_spmd

    nc = get_nc()
    in_maps = make_in_maps(inputs)
    res = run_bass_kernel_spmd(
        nc, in_maps, list(range(NCORES)), trace=trace, **kw
    )
    return gather_output(res.results, inputs["kernel"]), res


def kernel(**inputs) -> np.ndarray:
    out, _ = run_hw(inputs, trace=False)
    return out
